# revision 4
# baseline (speedup 1.0000x reference)
"""NoisyTopkRouter Trainium2 kernel.

Math (per batch b, data-parallel over 8 cores):
  h      = gelu(x @ W1 + b1)                         [T, H]
  scores = (h @ W2 + b2) @ TQ.T                      [T, E]
         = h @ (W2 @ TQ.T) + (b2 @ TQ.T)             (second matmul folded)
  gate   = sigmoid(x @ noise_w + noise_b)            [T, 1]
  noisy  = scores + temp * noise * gate
  top-2 -> masked softmax(noisy / (temp + 1e-6))

Device-side layout trick: x is shipped pre-transposed (xT [C, T]) so the
main matmul produces hT [H-slice, tok] tiles directly; the gate is computed
as an extra (33rd) H-slice of the same matmul whose activation is Sigmoid
instead of Gelu, and a one-hot column in the folded second-matmul weights
transposes it back to token-major for free.

f32r matmuls: full PE rate at ~1e-4 relative accuracy (vs 4x slower fp32).
"""

import numpy as np

import concourse.mybir as mybir
import concourse.tile as tile
from concourse import bacc
from concourse.bass_utils import run_bass_kernel_spmd
from concourse.masks import make_identity

B, T, C, E, TOPK = 8, 4096, 1024, 8, 2
H = 4 * C
HS_N = H // 128 + 1  # 33 H-slices: 32 real + 1 aug (gate)
HAUG = HS_N * 128  # 4224
KC_N = C // 128  # 8 k-chunks
TT = 512  # tokens per tile
TT_N = T // TT  # 8 token tiles
TS_N = TT // 128  # 4 token slices per tile
SW = 10  # stage-2 width: 8 experts + gate + pad (f32r needs even free dim)

F32 = mybir.dt.float32
F32R = mybir.dt.float32r
U32 = mybir.dt.uint32
AF = mybir.ActivationFunctionType
ALU = mybir.AluOpType

_CACHE = {}


def _build_nc(invtau):
    nc = bacc.Bacc(None, target_bir_lowering=False, debug=False)

    d_xT = nc.dram_tensor("xT", [C, T], F32R, kind="ExternalInput")
    d_w1 = nc.dram_tensor("w1aug", [C, HAUG], F32R, kind="ExternalInput")
    d_w2 = nc.dram_tensor("w2eaug", [HAUG, SW], F32R, kind="ExternalInput")
    d_b1 = nc.dram_tensor("b1aug", [HAUG], F32, kind="ExternalInput")
    d_sc = nc.dram_tensor("scoreconst", [128, E], F32, kind="ExternalInput")
    d_nz = nc.dram_tensor("noisepre", [T, E], F32, kind="ExternalInput")
    d_ro = nc.dram_tensor("router", [T, E], F32, kind="ExternalOutput")
    d_ix = nc.dram_tensor("topk", [T, TOPK], U32, kind="ExternalOutput")

    with tile.TileContext(nc) as tc:
        with (
            tc.tile_pool(name="res", bufs=1) as res,
            tc.tile_pool(name="xp", bufs=2) as xp,
            tc.tile_pool(name="hp", bufs=3) as hp,
            tc.tile_pool(name="np_", bufs=2) as npool,
            tc.tile_pool(name="sp", bufs=3) as sp,
            tc.tile_pool(name="op", bufs=2) as op,
            tc.tile_pool(name="psh", bufs=2, space="PSUM") as psh,
            tc.tile_pool(name="pst", bufs=2, space="PSUM") as pst,
            tc.tile_pool(name="ptp", bufs=2, space="PSUM") as ptp,
        ):
            # resident weights
            w1t = res.tile([128, KC_N, HAUG], F32R, tag="w1t")
            for kc in range(KC_N):
                nc.sync.dma_start(
                    w1t[:, kc, :], d_w1[kc * 128 : (kc + 1) * 128, :]
                )
            w2t = res.tile([128, HS_N, SW], F32R, tag="w2t")
            nc.sync.dma_start(w2t, d_w2.rearrange("(c p) e -> p c e", p=128))
            b1t = res.tile([128, HS_N], F32, tag="b1t")
            nc.sync.dma_start(b1t, d_b1.rearrange("(c p) -> p c", p=128))
            sct = res.tile([128, E], F32, tag="sct")
            nc.sync.dma_start(sct, d_sc[:])
            ident = res.tile([128, 128], F32, tag="ident")
            make_identity(nc, ident)

            for tt in range(TT_N):
                xt = xp.tile([128, KC_N, TT], F32R, tag="xt")
                nc.sync.dma_start(
                    xt,
                    d_xT.rearrange("(c p) t -> p c t", p=128)[
                        :, :, tt * TT : (tt + 1) * TT
                    ],
                )
                noiz = npool.tile([128, TS_N, E], F32, tag="noiz")
                nc.sync.dma_start(
                    noiz,
                    d_nz[tt * TT : (tt + 1) * TT, :].rearrange(
                        "(s p) e -> p s e", p=128
                    ),
                )

                psT = pst.tile([SW, TT], F32, tag="psT")
                for hs in range(HS_N):
                    ph = psh.tile([128, TT], F32, tag="psh")
                    for kc in range(KC_N):
                        nc.tensor.matmul(
                            ph,
                            w1t[:, kc, hs * 128 : (hs + 1) * 128],
                            xt[:, kc, :],
                            start=(kc == 0),
                            stop=(kc == KC_N - 1),
                        )
                    ht = hp.tile([128, TT], F32R, tag="ht")
                    nc.scalar.activation(
                        ht,
                        ph,
                        AF.Gelu if hs < HS_N - 1 else AF.Sigmoid,
                        bias=b1t[:, hs : hs + 1],
                    )
                    nc.tensor.matmul(
                        psT,
                        w2t[:, hs, :],
                        ht,
                        start=(hs == 0),
                        stop=(hs == HS_N - 1),
                    )

                sT = sp.tile([SW, TT], F32, tag="sT")
                nc.vector.tensor_copy(sT, psT)
                ptr = ptp.tile([128, TS_N, SW], F32, tag="ptr")
                for ts in range(TS_N):
                    nc.tensor.transpose(
                        ptr[:, ts, :],
                        sT[:, ts * 128 : (ts + 1) * 128],
                        ident[0:SW, 0:SW],
                    )

                rout = op.tile([128, TS_N, E], F32, tag="rout")
                idxo = op.tile([128, TS_N, TOPK], U32, tag="idxo")
                for ts in range(TS_N):
                    ps = ptr[:, ts, :]
                    # noisy = noise_pre * gate + scores + score_const
                    tmp = sp.tile([128, E], F32, tag="tmp")
                    nc.vector.scalar_tensor_tensor(
                        tmp,
                        noiz[:, ts, :],
                        ps[:, 8:9],
                        ps[:, 0:8],
                        op0=ALU.mult,
                        op1=ALU.add,
                    )
                    noisy = sp.tile([128, E], F32, tag="noisy")
                    nc.vector.tensor_add(noisy, tmp, sct)
                    m8 = sp.tile([128, 8], F32, tag="m8")
                    i8 = sp.tile([128, 8], U32, tag="i8")
                    nc.vector.max(out=m8, in_=noisy)
                    nc.vector.max_index(out=i8, in_max=m8, in_values=noisy)
                    nc.vector.tensor_copy(idxo[:, ts, :], i8[:, 0:TOPK])
                    # p2 = sigmoid((v2 - v1) * invtau); p1 = 1 - p2
                    nv1 = sp.tile([128, 1], F32, tag="nv1")
                    nc.vector.tensor_scalar_mul(nv1, m8[:, 0:1], -invtau)
                    p2 = sp.tile([128, 1], F32, tag="p2")
                    nc.scalar.activation(
                        p2, m8[:, 1:2], AF.Sigmoid, bias=nv1, scale=invtau
                    )
                    pd = sp.tile([128, 1], F32, tag="pd")
                    nc.vector.tensor_scalar(
                        pd, p2, -2.0, 1.0, op0=ALU.mult, op1=ALU.add
                    )
                    # out = (noisy>=v2)*p2 + (noisy>=v1)*(p1-p2)
                    a_t = sp.tile([128, E], F32, tag="a_t")
                    nc.vector.tensor_scalar(
                        a_t, noisy, m8[:, 1:2], p2, op0=ALU.is_ge, op1=ALU.mult
                    )
                    b_t = sp.tile([128, E], F32, tag="b_t")
                    nc.vector.tensor_scalar(
                        b_t, noisy, m8[:, 0:1], pd, op0=ALU.is_ge, op1=ALU.mult
                    )
                    nc.vector.tensor_add(rout[:, ts, :], a_t, b_t)

                nc.sync.dma_start(
                    d_ro[tt * TT : (tt + 1) * TT, :].rearrange(
                        "(s p) e -> p s e", p=128
                    ),
                    rout,
                )
                nc.sync.dma_start(
                    d_ix[tt * TT : (tt + 1) * TT, :].rearrange(
                        "(s p) k -> p s k", p=128
                    ),
                    idxo,
                )

    nc.compile()
    return nc


def _prep(x, noise, W1, b1, W2, b2, type_queries, noise_w, noise_b, temperature):
    temp = float(np.asarray(temperature))
    invtau = 1.0 / (temp + 1e-6)

    w1aug = np.zeros((C, HAUG), np.float32)
    w1aug[:, :H] = W1
    w1aug[:, H] = noise_w[:, 0]

    b1aug = np.zeros((HAUG,), np.float32)
    b1aug[:H] = b1
    b1aug[H] = noise_b[0]

    w2eaug = np.zeros((HAUG, SW), np.float32)
    w2eaug[:H, :E] = (W2.astype(np.float64) @ type_queries.astype(np.float64).T).astype(
        np.float32
    )
    w2eaug[H, 8] = 1.0

    score_const = (b2.astype(np.float64) @ type_queries.astype(np.float64).T).astype(
        np.float32
    )
    sct = np.broadcast_to(score_const[None, :], (128, E)).copy()

    noise_pre = (temp * np.asarray(noise, np.float64)).astype(np.float32)

    in_maps = []
    for b in range(B):
        in_maps.append(
            {
                "xT": np.ascontiguousarray(np.asarray(x[b], np.float32).T),
                "w1aug": w1aug,
                "w2eaug": w2eaug,
                "b1aug": b1aug,
                "scoreconst": sct,
                "noisepre": noise_pre[b],
            }
        )
    return invtau, in_maps


def _run(inputs, trace=False):
    invtau, in_maps = _prep(**inputs)
    key = round(invtau, 9)
    if key not in _CACHE:
        _CACHE[key] = _build_nc(invtau)
    nc = _CACHE[key]
    res = run_bass_kernel_spmd(
        nc, in_maps, core_ids=list(range(B)), trace=trace
    )
    router = np.stack([r["router"] for r in res.results]).astype(np.float32)
    topk = np.stack([r["topk"] for r in res.results]).astype(np.int32)
    return (router, topk), res


def kernel(**inputs):
    out, _ = _run(inputs, trace=False)
    return out


# revision 8
# speedup vs baseline: 1.7299x; 1.7299x over previous
"""NoisyTopkRouter Trainium2 kernel.

Math (per batch b, data-parallel over 8 cores):
  h      = gelu(x @ W1 + b1)                         [T, H]
  scores = (h @ W2 + b2) @ TQ.T                      [T, E]
         = h @ (W2 @ TQ.T) + (b2 @ TQ.T)             (second matmul folded)
  gate   = sigmoid(x @ noise_w + noise_b)            [T, 1]
  noisy  = scores + temp * noise * gate
  top-2 -> masked softmax(noisy / (temp + 1e-6))

Device-side layout: x is shipped pre-transposed (xT [C, T]) so the main
matmul produces hT [H-slice, tok] tiles directly; the gate is computed as an
extra (33rd) H-slice of the same matmul whose activation is Sigmoid instead
of Gelu. Scores accumulate transposed ([10, tok], W2E stationary - cheap
weight loads), then 128-wide blocks are PE-transposed back to token-major.
W1 is streamed per-H-slice so the first matmul starts after ~2.5MB of DMA
instead of 17MB. Each tile's post-processing is deferred into the next
tile's matmul stream to keep the PE dense.

f32r matmuls: full PE rate at ~1e-4 relative accuracy (vs 4x slower fp32).
"""

import contextlib

import numpy as np

import concourse.mybir as mybir
import concourse.tile as tile
from concourse import bacc
from concourse.bass_utils import run_bass_kernel_spmd
from concourse.masks import make_identity

B, T, C, E, TOPK = 8, 4096, 1024, 8, 2
H = 4 * C
HS_N = H // 128 + 1  # 33 H-slices: 32 real + 1 aug (gate)
HAUG = HS_N * 128  # 4224
KC_N = C // 128  # 8 k-chunks
TT = 512  # tokens per tile
TT_N = T // TT  # 8 token tiles
TS_N = TT // 128  # 4 token slices per tile
SW = 10  # stage-2 width: 8 experts + gate + pad (f32r needs even free dim)

F32 = mybir.dt.float32
F32R = mybir.dt.float32r
U32 = mybir.dt.uint32
AF = mybir.ActivationFunctionType
ALU = mybir.AluOpType

_CACHE = {}


def _build_nc(invtau, reps=1):
    nc = bacc.Bacc(None, target_bir_lowering=False, debug=False)

    d_xT = nc.dram_tensor("xT", [C, T], F32R, kind="ExternalInput")
    d_w1 = nc.dram_tensor("w1r", [128, HS_N, KC_N, 128], F32R, kind="ExternalInput")
    d_w2 = nc.dram_tensor("w2eaug", [HAUG, SW], F32R, kind="ExternalInput")
    d_b1 = nc.dram_tensor("b1aug", [HAUG], F32, kind="ExternalInput")
    d_nz = nc.dram_tensor("noisepk", [T, 2, E], F32, kind="ExternalInput")
    d_ro = nc.dram_tensor("router", [T, E], F32, kind="ExternalOutput")
    d_ix = nc.dram_tensor("topk", [T, TOPK], U32, kind="ExternalOutput")

    with tile.TileContext(nc) as tc:
        with (
            tc.tile_pool(name="res", bufs=1) as res,
            tc.tile_pool(name="xp", bufs=2) as xp,
            tc.tile_pool(name="hp", bufs=3) as hp,
            tc.tile_pool(name="np_", bufs=2) as npool,
            tc.tile_pool(name="sp", bufs=3) as sp,
            tc.tile_pool(name="op", bufs=2) as op,
            tc.tile_pool(name="psh", bufs=2, space="PSUM") as psh,
            tc.tile_pool(name="pst", bufs=2, space="PSUM") as pst,
            tc.tile_pool(name="ptp", bufs=2, space="PSUM") as ptp,
        ):
            env = {}
            # small resident constants first (cheap DMAs)
            w2t = res.tile([128, HS_N, SW], F32R, tag="w2t")
            nc.sync.dma_start(w2t, d_w2.rearrange("(c p) e -> p c e", p=128))
            b1t = res.tile([128, HS_N], F32, tag="b1t")
            nc.sync.dma_start(b1t, d_b1.rearrange("(c p) -> p c", p=128))
            ident = res.tile([128, 128], F32, tag="ident")
            make_identity(nc, ident)
            # prefetch tile-0 inputs BEFORE the bulk W1 stream so the first
            # matmul isn't queued behind 17MB of weights
            xt0 = xp.tile([128, KC_N, TT], F32R, tag="xt")
            nc.sync.dma_start(
                xt0, d_xT.rearrange("(c p) t -> p c t", p=128)[:, :, 0:TT]
            )
            noiz0 = npool.tile([128, TS_N, 2, E], F32, tag="noiz")
            nc.sync.dma_start(
                noiz0, d_nz[0:TT, :, :].rearrange("(s p) k e -> p s k e", p=128)
            )
            # W1: per-H-slice DMAs (512KB each, 4KB/partition contiguous)
            # so hs=0 weights land quickly and the PE can start early.
            w1t = res.tile([128, HS_N, KC_N, 128], F32R, tag="w1t")
            for hs in range(HS_N):
                nc.sync.dma_start(w1t[:, hs], d_w1[:, hs])

            env.update(
                xt0=xt0, noiz0=noiz0,
                w1t=w1t, w2t=w2t, b1t=b1t, ident=ident,
                xp=xp, hp=hp, npool=npool, sp=sp, op=op,
                psh=psh, pst=pst, ptp=ptp,
                d_xT=d_xT, d_nz=d_nz, d_ro=d_ro, d_ix=d_ix,
                invtau=invtau,
            )
            rep_ctx = tc.For_i(0, reps, 1) if reps > 1 else contextlib.nullcontext()
            with rep_ctx:
                _tt_loop(nc, env)

    nc.compile()
    return nc


def _tt_loop(nc, env):
    xp, hp, npool = env["xp"], env["hp"], env["npool"]
    psh, pst = env["psh"], env["pst"]
    w1t, w2t, b1t = env["w1t"], env["w2t"], env["b1t"]
    d_xT, d_nz = env["d_xT"], env["d_nz"]

    pending = None
    for tt in range(TT_N):
        if tt == 0:
            xt = env["xt0"]
            noiz = env["noiz0"]
        else:
            xt = xp.tile([128, KC_N, TT], F32R, tag="xt")
            nc.sync.dma_start(
                xt,
                d_xT.rearrange("(c p) t -> p c t", p=128)[
                    :, :, tt * TT : (tt + 1) * TT
                ],
            )
            noiz = npool.tile([128, TS_N, 2, E], F32, tag="noiz")
            nc.sync.dma_start(
                noiz,
                d_nz[tt * TT : (tt + 1) * TT, :, :].rearrange(
                    "(s p) k e -> p s k e", p=128
                ),
            )

        psT = pst.tile([SW, TT], F32, tag="psT")
        for hs in range(HS_N):
            ph = psh.tile([128, TT], F32, tag="psh")
            for kc in range(KC_N):
                nc.tensor.matmul(
                    ph,
                    w1t[:, hs, kc, :],
                    xt[:, kc, :],
                    start=(kc == 0),
                    stop=(kc == KC_N - 1),
                )
            ht = hp.tile([128, TT], F32R, tag="ht")
            # gate slice: sigmoid(z) = 0.5 + 0.5*tanh(z/2); tanh shares the
            # gelu table set, so the whole kernel needs one ACT table load
            nc.scalar.activation(
                ht,
                ph,
                AF.Gelu if hs < HS_N - 1 else AF.Tanh,
                bias=b1t[:, hs : hs + 1],
                scale=1.0 if hs < HS_N - 1 else 0.5,
            )
            nc.tensor.matmul(
                psT, w2t[:, hs, :], ht, start=(hs == 0), stop=(hs == HS_N - 1)
            )
            # previous tile's post-processing, emitted inside this tile's
            # matmul stream so the PE never idles at the boundary
            if hs == 1 and pending is not None:
                _post(nc, env, *pending)
                pending = None
        sT = env["sp"].tile([SW, TT], F32, tag="sT")
        nc.vector.tensor_copy(sT, psT)
        pending = (sT, noiz, tt)
    _post(nc, env, *pending)


def _post(nc, env, sT, noiz, tt):
    sp, op, ptp = env["sp"], env["op"], env["ptp"]
    ident, invtau = env["ident"], env["invtau"]
    d_ro, d_ix = env["d_ro"], env["d_ix"]

    ptr = ptp.tile([128, TS_N, SW], F32, tag="ptr")
    for ts in range(TS_N):
        nc.tensor.transpose(
            ptr[:, ts, :], sT[:, ts * 128 : (ts + 1) * 128], ident[0:SW, 0:SW]
        )

    rout = op.tile([128, TS_N, E], F32, tag="rout")
    idxo = op.tile([128, TS_N, TOPK], U32, tag="idxo")
    for ts in range(TS_N):
        ps = ptr[:, ts, :]
        # tanh_gate t is in ps[:, 8]; gate = 0.5 + 0.5*t, and the host packed
        # noiz[...,0,:] = 0.5*temp*noise, noiz[...,1,:] = 0.5*temp*noise + b2@TQ.T
        # so noisy = scores + noiz0*t + noiz1
        tmp = sp.tile([128, E], F32, tag="tmp")
        nc.vector.scalar_tensor_tensor(
            tmp,
            noiz[:, ts, 0, :],
            ps[:, 8:9],
            ps[:, 0:8],
            op0=ALU.mult,
            op1=ALU.add,
        )
        noisy = sp.tile([128, E], F32, tag="noisy")
        nc.vector.tensor_add(noisy, tmp, noiz[:, ts, 1, :])
        m8 = sp.tile([128, 8], F32, tag="m8")
        i8 = sp.tile([128, 8], U32, tag="i8")
        nc.vector.max(out=m8, in_=noisy)
        nc.vector.max_index(out=i8, in_max=m8, in_values=noisy)
        nc.vector.tensor_copy(idxo[:, ts, :], i8[:, 0:TOPK])
        # p2 = sigmoid((v2-v1)*invtau) = 0.5 + 0.5*tanh((v2-v1)*invtau/2)
        nv1 = sp.tile([128, 1], F32, tag="nv1")
        nc.vector.tensor_scalar_mul(nv1, m8[:, 0:1], -0.5 * invtau)
        th = sp.tile([128, 1], F32, tag="th")
        nc.scalar.activation(th, m8[:, 1:2], AF.Tanh, bias=nv1, scale=0.5 * invtau)
        p2 = sp.tile([128, 1], F32, tag="p2")
        nc.vector.tensor_scalar(p2, th, 0.5, 0.5, op0=ALU.mult, op1=ALU.add)
        # p1 - p2 = -tanh
        pd = sp.tile([128, 1], F32, tag="pd")
        nc.vector.tensor_scalar_mul(pd, th, -1.0)
        # out = (noisy>=v2)*p2 + (noisy>=v1)*(p1-p2)
        a_t = sp.tile([128, E], F32, tag="a_t")
        nc.vector.tensor_scalar(
            a_t, noisy, m8[:, 1:2], p2, op0=ALU.is_ge, op1=ALU.mult
        )
        b_t = sp.tile([128, E], F32, tag="b_t")
        nc.vector.tensor_scalar(
            b_t, noisy, m8[:, 0:1], pd, op0=ALU.is_ge, op1=ALU.mult
        )
        nc.vector.tensor_add(rout[:, ts, :], a_t, b_t)

    nc.sync.dma_start(
        d_ro[tt * TT : (tt + 1) * TT, :].rearrange("(s p) e -> p s e", p=128), rout
    )
    nc.sync.dma_start(
        d_ix[tt * TT : (tt + 1) * TT, :].rearrange("(s p) k -> p s k", p=128), idxo
    )


def _prep(x, noise, W1, b1, W2, b2, type_queries, noise_w, noise_b, temperature):
    temp = float(np.asarray(temperature))
    invtau = 1.0 / (temp + 1e-6)

    w1aug = np.zeros((C, HAUG), np.float32)
    w1aug[:, :H] = W1
    w1aug[:, H] = np.asarray(noise_w)[:, 0]
    # [kc*128+p, hs*128+m] -> [p, hs, kc, m]
    w1r = np.ascontiguousarray(
        w1aug.reshape(KC_N, 128, HS_N, 128).transpose(1, 2, 0, 3)
    )

    b1aug = np.zeros((HAUG,), np.float32)
    b1aug[:H] = b1
    # gate slice computes tanh((z + noise_b)/2) = tanh(z*0.5 + noise_b*0.5)
    b1aug[H] = 0.5 * float(np.asarray(noise_b)[0])

    w2eaug = np.zeros((HAUG, SW), np.float32)
    w2eaug[:H, :E] = (
        np.asarray(W2, np.float64) @ np.asarray(type_queries, np.float64).T
    ).astype(np.float32)
    w2eaug[H, 8] = 1.0

    score_const = (
        np.asarray(b2, np.float64) @ np.asarray(type_queries, np.float64).T
    ).astype(np.float64)

    half_noise = 0.5 * temp * np.asarray(noise, np.float64)  # [B, T, E]
    noise_pk = np.empty((B, T, 2, E), np.float32)
    noise_pk[:, :, 0, :] = half_noise
    noise_pk[:, :, 1, :] = half_noise + score_const[None, None, :]

    in_maps = []
    for b in range(B):
        in_maps.append(
            {
                "xT": np.ascontiguousarray(np.asarray(x[b], np.float32).T),
                "w1r": w1r,
                "w2eaug": w2eaug,
                "b1aug": b1aug,
                "noisepk": noise_pk[b],
            }
        )
    return invtau, in_maps


def _run(inputs, trace=False):
    invtau, in_maps = _prep(**inputs)
    key = round(invtau, 9)
    if key not in _CACHE:
        _CACHE[key] = _build_nc(invtau)
    nc = _CACHE[key]
    res = run_bass_kernel_spmd(nc, in_maps, core_ids=list(range(B)), trace=trace)
    router = np.stack([r["router"] for r in res.results]).astype(np.float32)
    topk = np.stack([r["topk"] for r in res.results]).astype(np.int32)
    return (router, topk), res


def kernel(**inputs):
    out, _ = _run(inputs, trace=False)
    return out
